# revision 12
# baseline (speedup 1.0000x reference)
"""Trainium2 Bass kernel for nn_BlockTransformer_80092550136042.

12-layer pre-LN block transformer (B=8, N=1296 tokens, D=768, 12 heads,
FFN 3072) with a static block-sparse attention mask.

Strategy:
  - Data-parallel: one batch element per NeuronCore (8 cores, no collectives).
  - All matmuls in float32r (TF32-like: 1 cycle/row at free-dim >= 256,
    ~1e-4 rounding) with fp32 PSUM accumulation.
  - Token-major fp32 residual stream resident in SBUF; LN outputs are
    PE-transposed to feature-major for the projections.
  - Attention computed transposed (scores S^T[k,q]) so softmax denominators
    come from an appended ones-column in V (no partition reductions); softmax
    skips max-subtraction (scores are provably tiny for this model family) and
    applies the mask multiplicatively after exp.
  - LN scale/bias folded into the following projection weights host-side.
  - Static rule-mask block sparsity: fully-masked (kt,strip) blocks skipped,
    partial blocks multiplied by streamed 0/1 rule tiles.
"""
import os
import sys
import numpy as np
from contextlib import ExitStack


def _to_bf16(a):
    import ml_dtypes
    return np.ascontiguousarray(a.astype(ml_dtypes.bfloat16))

# ---------------- problem constants (hardcoded from the spec) ----------------
B, P0, H, NO, NA = 8, 16, 32, 32, 8
D, L, NH, F = 768, 12, 12, 3072
DH = D // NH                      # 64
NQ = P0 + H * (NO + NA)           # 1296 real tokens
KT = 11                           # token tiles of 128
NT = KT * 128                     # 1408 padded tokens
DK = D // 128                     # 6
FK = F // 128                     # 24
FCH = 6                           # FFN chunks (512 F-rows each)
FCK = FK // FCH                   # 4 k-tiles per FFN chunk
STRIPS = [(0, 512), (512, 512), (1024, NQ - 1024)]   # q strips (<=512, last 272)
SCALE = np.float32(1.0 / np.sqrt(DH))
EPS = 1e-6

for _p in ('/opt/trn_rl_repo',):
    if _p not in sys.path and os.path.isdir(_p):
        sys.path.append(_p)


def _rule_mask_np():
    """Static attention rule mask, [q, k] bool (copied from the model spec)."""
    g = np.concatenate([np.zeros(P0, np.int32),
                        np.tile(np.concatenate([np.full(NO, 1, np.int32),
                                                np.full(NA, 2, np.int32)]), H)])
    t = np.concatenate([np.full(P0, -1, np.int32),
                        np.repeat(np.arange(H, dtype=np.int32), NO + NA)])
    ti, tj = t[:, None], t[None, :]
    rules = {(0, 0): 'all', (1, 0): 'all', (2, 0): 'all',
             (1, 1): 'causal', (2, 1): 'causal', (2, 2): 'causal'}
    m = np.zeros((NQ, NQ), bool)
    for (gi, gj), r in rules.items():
        sel = (g[:, None] == gi) & (g[None, :] == gj)
        cond = np.ones((NQ, NQ), bool) if r == 'all' else (tj <= ti)
        m |= sel & cond
    return m


def _block_structure():
    """Per (kt tile, strip): 'skip' / 'full' / 'partial' from the rule mask."""
    rule_t = np.zeros((NT, NQ), np.float32)          # [k_token, q]
    rule_t[:NQ, :] = _rule_mask_np().T.astype(np.float32)
    ks_lists, partial = {}, {}
    for si, (q0, w) in enumerate(STRIPS):
        ks = []
        for kt in range(KT):
            blk = rule_t[kt * 128:(kt + 1) * 128, q0:q0 + w]
            if blk.any():
                ks.append(kt)
                partial[(kt, si)] = not blk.all()
        ks_lists[si] = ks
    return rule_t, ks_lists, partial


_NC_CACHE = {}
_LAST_RESULTS = None


def _build(n_layers, need_pad):
    import concourse.bacc as bacc
    import concourse.mybir as mybir
    import concourse.tile as tile

    f32 = mybir.dt.float32
    f32r = mybir.dt.float32r
    bf16 = mybir.dt.bfloat16
    AF = mybir.ActivationFunctionType
    AX = mybir.AxisListType

    _, ks_lists, partial = _block_structure()

    nc = bacc.Bacc(None, target_bir_lowering=False)
    with tile.TileContext(nc) as tc, ExitStack() as ctx:
        dram = ctx.enter_context(tc.tile_pool(name="dram", bufs=1, space="DRAM"))

        def din(shape, dtype, name):
            return dram.tile(shape, dtype, kind="ExternalInput", name=name,
                             uniquify=False)

        x0_d = din([NT, D], f32, "x0")
        wq_d = din([L, D, D], bf16, "wq")
        wk_d = din([L, D, D], bf16, "wk")
        wv_d = din([L, D, D], bf16, "wv")
        wo_d = din([L, D, D], bf16, "wo")
        w1_d = din([L, D, F], bf16, "w1")
        w2_d = din([L, F, D], bf16, "w2")
        bqp_d = din([128, L * DK], f32, "bqp")
        bkp_d = din([128, L * DK], f32, "bkp")
        b1p_d = din([128, L * FK], f32, "b1p")
        bvr_d = din([L, D], bf16, "bvr")
        bor_d = din([L, D], bf16, "bor")
        b2r_d = din([L, D], bf16, "b2r")
        rul_d = din([NT, NQ], bf16, "rul")
        idn_d = din([128, 128], bf16, "idn")
        ones_d = din([1, 128], bf16, "ones")
        onesr_d = din([1, 128], f32r, "onesr")
        vones_d = din([128, 12], f32r, "vones")
        lnfs_d = din([1, D], f32r, "lnfs")
        lnfb_d = din([1, D], f32r, "lnfb")
        padc_d = din([128, KT], f32, "padc") if need_pad else None
        out_d = dram.tile([NT, D], f32, kind="ExternalOutput", name="out",
                          uniquify=False)

        const = ctx.enter_context(tc.tile_pool(name="const", bufs=1))
        idn_sb = const.tile([128, 128], bf16, name="idn_sb")
        nc.sync.dma_start(idn_sb[:], idn_d[:])
        ones_sb = const.tile([1, 128], bf16, name="ones_sb")
        nc.sync.dma_start(ones_sb[:], ones_d[:])
        onesr_sb = const.tile([1, 128], f32r, name="onesr_sb")
        nc.sync.dma_start(onesr_sb[:], onesr_d[:])
        bqp_sb = const.tile([128, L * DK], f32, name="bqp_sb")
        nc.sync.dma_start(bqp_sb[:], bqp_d[:])
        bkp_sb = const.tile([128, L * DK], f32, name="bkp_sb")
        nc.sync.dma_start(bkp_sb[:], bkp_d[:])
        b1p_sb = const.tile([128, L * FK], f32, name="b1p_sb")
        nc.sync.dma_start(b1p_sb[:], b1p_d[:])
        if need_pad:
            padc_sb = const.tile([128, KT], f32, name="padc_sb")
            nc.sync.dma_start(padc_sb[:], padc_d[:])
        eps_sb = const.tile([128, 1], f32, name="eps_sb")
        nc.any.memset(eps_sb[:], float(EPS))

        big = ctx.enter_context(tc.tile_pool(name="big", bufs=1))
        rule_sb = big.tile([128, KT, NQ], bf16, name="rule_sb")
        nc.sync.dma_start(rule_sb[:],
                          rul_d[:].rearrange("(t p) q -> p t q", p=128))
        x_sb = big.tile([128, KT, D], f32, name="x_sb")
        nc.sync.dma_start(x_sb[:], x0_d[:].rearrange("(t p) d -> p t d", p=128))

        rowp = ctx.enter_context(tc.tile_pool(name="rowp", bufs=2))
        statp = ctx.enter_context(tc.tile_pool(name="statp", bufs=3))

        def emit_ln_transpose(lidx, dstT, tpool, tpsum):
            """LN (normalize only) of x then PE-transpose into dstT [128,DK,NT]."""
            for t in range(KT):
                xt = x_sb[:, t, :]
                mu = statp.tile([128, 1], f32, name="mu", tag="mu")
                nc.vector.reduce_sum(mu[:], xt, axis=AX.X)
                nc.vector.tensor_scalar_mul(mu[:], mu[:], 1.0 / D)
                xc = tpool.tile([128, D], f32, name="xc", tag="xc")
                nc.vector.tensor_scalar_sub(xc[:], xt, mu[:])
                sq = tpool.tile([128, D], f32, name="sq", tag="sq")
                ss = statp.tile([128, 1], f32, name="ss", tag="ss")
                nc.scalar.activation(sq[:], xc[:], AF.Square, accum_out=ss[:])
                std = statp.tile([128, 1], f32, name="std", tag="std")
                nc.scalar.activation(std[:], ss[:], AF.Sqrt, bias=eps_sb[:],
                                     scale=1.0 / D)
                rstd = statp.tile([128, 1], f32, name="rstd", tag="rstd")
                nc.vector.reciprocal(rstd[:], std[:])
                zt = tpool.tile([128, D], bf16, name="zt", tag="zt")
                nc.vector.tensor_scalar_mul(zt[:], xc[:], rstd[:])
                for ci in range(DK):
                    tp = tpsum.tile([128, 128], bf16, name="tp", tag="tp")
                    nc.tensor.transpose(tp[:], zt[:, ci * 128:(ci + 1) * 128],
                                        idn_sb[:])
                    nc.any.tensor_copy(dstT[:, ci, t * 128:(t + 1) * 128], tp[:])

        def emit_layer(l):
            zpool = ctx_l.enter_context(tc.tile_pool(name=f"z{l}", bufs=1))
            zT = zpool.tile([128, DK, NT], bf16, name=f"zT{l}", tag="zT")
            with tc.tile_pool(name=f"lnp{l}a", bufs=3) as lnp, \
                 tc.tile_pool(name=f"lnps{l}a", bufs=3, space="PSUM") as lnps:
                emit_ln_transpose(l, zT, lnp, lnps)
            akp = ctx_a.enter_context(tc.tile_pool(name=f"akv{l}", bufs=1))
            qT = akp.tile([128, DK, NT], bf16, name=f"qT{l}", tag="qT")
            kT = akp.tile([128, DK, NT], bf16, name=f"kT{l}", tag="kT")
            # k^T is only produced for real tokens; ST for the last kt tile
            # reads through col NT, so zero the pad-token columns (their
            # exp(0)=1 scores are killed by the rule mask).
            nc.any.memset(kT[:, :, NQ:NT], 0.0)
            v65 = akp.tile([128, KT, 12 * 65], f32r, name=f"v65{l}", tag="v65")
            v65v = v65[:].rearrange("p t (h c) -> p t h c", h=12)
            for t in range(KT):
                nc.sync.dma_start(v65v[:, t, :, 64], vones_d[:])

            # ---- QKV projections ----
            with tc.tile_pool(name=f"wp{l}", bufs=8) as wpool, \
                 tc.tile_pool(name=f"qps{l}", bufs=3, space="PSUM") as qps:
                for kind, (w_dr, dstT, biasp) in enumerate(
                        [(wq_d, qT, bqp_sb), (wk_d, kT, bkp_sb)]):
                    wsl = []
                    for k in range(DK):
                        wt = wpool.tile([128, D], bf16, name=f"w{kind}{l}{k}",
                                        tag="wsl")
                        nc.sync.dma_start(wt[:], w_dr[l, k * 128:(k + 1) * 128, :])
                        wsl.append(wt)
                    for m in range(DK):
                        for (q0, w) in STRIPS:
                            ps = qps.tile([128, 512], f32, name="pp", tag="pp")
                            for ki in range(DK):
                                nc.tensor.matmul(
                                    ps[:, :w],
                                    lhsT=wsl[ki][:, m * 128:(m + 1) * 128],
                                    rhs=zT[:, ki, q0:q0 + w],
                                    start=(ki == 0), stop=(ki == DK - 1))
                            nc.scalar.activation(
                                dstT[:, m, q0:q0 + w], ps[:, :w], AF.Identity,
                                bias=biasp[:, l * DK + m:l * DK + m + 1])
                # V projection (token-major, bias via augmented-K ones row)
                bvrow = rowp.tile([1, D], bf16, name=f"bv{l}", tag="brow")
                nc.sync.dma_start(bvrow[:], bvr_d[l:l + 1, :])
                wsl = []
                for k in range(DK):
                    wt = wpool.tile([128, D], bf16, name=f"wv{l}{k}", tag="wsl")
                    nc.sync.dma_start(wt[:], wv_d[l, k * 128:(k + 1) * 128, :])
                    wsl.append(wt)
                for t in range(KT):
                    for hf in range(2):
                        ps = qps.tile([128, 384], f32, name="pv", tag="pv")
                        for ki in range(DK):
                            nc.tensor.matmul(
                                ps[:],
                                lhsT=zT[:, ki, t * 128:(t + 1) * 128],
                                rhs=wsl[ki][:, hf * 384:(hf + 1) * 384],
                                start=(ki == 0), stop=False)
                        nc.tensor.matmul(ps[:], lhsT=ones_sb[0:1, 0:128],
                                         rhs=bvrow[0:1, hf * 384:(hf + 1) * 384],
                                         start=False, stop=True)
                        nc.any.tensor_copy(
                            v65v[:, t, hf * 6:(hf + 1) * 6, 0:64],
                            ps[:].rearrange("p (h c) -> p h c", h=6))

            # ---- attention + O-projection, strip-major ----
            borow = rowp.tile([1, D], bf16, name=f"bo{l}", tag="brow")
            nc.sync.dma_start(borow[:], bor_d[l:l + 1, :])
            with tc.tile_pool(name=f"wo{l}", bufs=7) as wop, \
                 tc.tile_pool(name=f"ap{l}", bufs=6) as apool, \
                 tc.tile_pool(name=f"ao{l}", bufs=2) as opool, \
                 tc.tile_pool(name=f"rz{l}", bufs=4) as rzp, \
                 tc.tile_pool(name=f"stp{l}", bufs=4, space="PSUM") as stps, \
                 tc.tile_pool(name=f"avp{l}", bufs=2, space="PSUM") as avps, \
                 tc.tile_pool(name=f"ops{l}", bufs=2, space="PSUM") as ops:
                wo_sl = []
                for k in range(DK):
                    wt = wop.tile([128, D], bf16, name=f"wo{l}{k}", tag="wo")
                    nc.sync.dma_start(wt[:], wo_d[l, k * 128:(k + 1) * 128, :])
                    wo_sl.append(wt)
                for si, (q0, w) in enumerate(STRIPS):
                    ks = ks_lists[si]
                    oTs = opool.tile([128, DK, 512], bf16, name=f"oTs{l}{si}",
                                     tag="oTs")
                    for p in range(DK):
                        avA = avps.tile([65, 512], f32, name="avA", tag="av")
                        avB = avps.tile([65, 512], f32, name="avB", tag="av")
                        for j, kt in enumerate(ks):
                            pts = []
                            for hh, (r0, r1) in enumerate(((0, 64), (64, 128))):
                                st = stps.tile([128, 512], f32, name="st",
                                               tag="st")
                                nc.tensor.matmul(
                                    st[:, :w],
                                    lhsT=kT[r0:r1, p, kt * 128:(kt + 1) * 128],
                                    rhs=qT[r0:r1, p, q0:q0 + w],
                                    start=True, stop=True)
                                pt = apool.tile([128, 512], f32r, name="pt",
                                                tag="pt")
                                nc.scalar.activation(pt[:, :w], st[:, :w], AF.Exp)
                                if partial[(kt, si)]:
                                    nc.vector.tensor_mul(
                                        pt[:, :w], pt[:, :w],
                                        rule_sb[:, kt, q0:q0 + w])
                                if need_pad:
                                    nc.vector.tensor_scalar_mul(
                                        pt[:, :w], pt[:, :w],
                                        padc_sb[:, kt:kt + 1])
                                pts.append(pt)
                            first, last = (j == 0), (j == len(ks) - 1)
                            nc.tensor.matmul(avA[:, :w],
                                             lhsT=v65v[:, kt, 2 * p, :],
                                             rhs=pts[0][:, :w],
                                             start=first, stop=last)
                            nc.tensor.matmul(avB[:, :w],
                                             lhsT=v65v[:, kt, 2 * p + 1, :],
                                             rhs=pts[1][:, :w],
                                             start=first, stop=last)
                        for hh, av in ((0, avA), (1, avB)):
                            rz = rzp.tile([1, 512], f32r, name="rz", tag="rz")
                            with nc.allow_low_precision(
                                    reason="1/Z broadcast operand is f32r"):
                                nc.vector.reciprocal(rz[:, :w], av[64:65, :w])
                            bc = stps.tile([64, 512], f32, name="bc", tag="st")
                            nc.tensor.matmul(bc[:, :w], lhsT=onesr_sb[0:1, 0:64],
                                             rhs=rz[0:1, :w],
                                             start=True, stop=True)
                            bcs = rzp.tile([64, 512], f32, name="bcs", tag="bcs")
                            nc.any.tensor_copy(bcs[:, :w], bc[:, :w])
                            nc.vector.tensor_mul(
                                oTs[64 * hh:64 * hh + 64, p, :w],
                                av[0:64, :w], bcs[:, :w])
                    # O-projection for this strip's token windows
                    t0 = q0 // 128
                    nwin = (w + 127) // 128
                    for wi in range(nwin):
                        t = t0 + wi
                        lc = wi * 128
                        tw = min(128, w - lc)
                        for hf in range(2):
                            pso = ops.tile([128, 384], f32, name="po", tag="po")
                            for ki in range(DK):
                                nc.tensor.matmul(
                                    pso[:tw, :],
                                    lhsT=oTs[:, ki, lc:lc + tw],
                                    rhs=wo_sl[ki][:, hf * 384:(hf + 1) * 384],
                                    start=(ki == 0), stop=False)
                            nc.tensor.matmul(
                                pso[:tw, :], lhsT=ones_sb[0:1, 0:tw],
                                rhs=borow[0:1, hf * 384:(hf + 1) * 384],
                                start=False, stop=True)
                            nc.vector.tensor_add(
                                x_sb[0:tw, t, hf * 384:(hf + 1) * 384],
                                x_sb[0:tw, t, hf * 384:(hf + 1) * 384],
                                pso[:tw, :])

            ctx_a.close()
            # ---- LN2 + FFN ----
            zT2 = zpool.tile([128, DK, NT], bf16, name=f"zT2{l}", tag="zT")
            with tc.tile_pool(name=f"lnp{l}b", bufs=3) as lnp, \
                 tc.tile_pool(name=f"lnps{l}b", bufs=3, space="PSUM") as lnps:
                emit_ln_transpose(l, zT2, lnp, lnps)

            b2row = rowp.tile([1, D], bf16, name=f"b2{l}", tag="brow")
            nc.sync.dma_start(b2row[:], b2r_d[l:l + 1, :])
            with tc.tile_pool(name=f"w1p{l}", bufs=8) as w1p, \
                 tc.tile_pool(name=f"w2p{l}", bufs=6) as w2p, \
                 tc.tile_pool(name=f"atp{l}", bufs=2) as atp, \
                 tc.tile_pool(name=f"gsc{l}", bufs=2) as gsc, \
                 tc.tile_pool(name=f"f1ps{l}", bufs=3, space="PSUM") as f1ps, \
                 tc.tile_pool(name=f"f2ps{l}", bufs=3, space="PSUM") as f2ps:
                for fc in range(FCH):
                    aT = atp.tile([128, FCK, NQ], bf16, name=f"aT{l}{fc}",
                                  tag="aT")
                    w1sl = []
                    for k in range(DK):
                        wt = w1p.tile([128, 512], bf16, name=f"w1{l}{fc}{k}",
                                      tag="w1")
                        nc.sync.dma_start(
                            wt[:], w1_d[l, k * 128:(k + 1) * 128,
                                        fc * 512:(fc + 1) * 512])
                        w1sl.append(wt)
                    CG = float(np.sqrt(2.0 / np.pi))
                    for mf in range(FCK):
                        fm = fc * FCK + mf
                        for (q0, w) in STRIPS:
                            ps = f1ps.tile([128, 512], f32, name="p1", tag="p1")
                            for ki in range(DK):
                                nc.tensor.matmul(
                                    ps[:, :w],
                                    lhsT=w1sl[ki][:, mf * 128:(mf + 1) * 128],
                                    rhs=zT2[:, ki, q0:q0 + w],
                                    start=(ki == 0), stop=(ki == DK - 1))
                            # gelu(tanh approx) from primitives:
                            # u = psum + b1 ; g = 0.5*u*(1+tanh(C*(u+0.044715*u^3)))
                            u = gsc.tile([128, 512], f32, name="gu", tag="gu")
                            nc.scalar.activation(
                                u[:, :w], ps[:, :w], AF.Identity,
                                bias=b1p_sb[:, l * FK + fm:l * FK + fm + 1])
                            s = gsc.tile([128, 512], f32, name="gs", tag="gs")
                            nc.scalar.activation(s[:, :w], u[:, :w], AF.Square)
                            nc.vector.tensor_scalar(
                                s[:, :w], s[:, :w], 0.044715 * CG, CG,
                                mybir.AluOpType.mult, mybir.AluOpType.add)
                            nc.vector.tensor_mul(s[:, :w], s[:, :w], u[:, :w])
                            th = gsc.tile([128, 512], f32, name="gt", tag="gt")
                            nc.scalar.activation(th[:, :w], s[:, :w], AF.Tanh)
                            nc.vector.tensor_scalar(
                                th[:, :w], th[:, :w], 1.0, 0.5,
                                mybir.AluOpType.add, mybir.AluOpType.mult)
                            nc.vector.tensor_mul(aT[:, mf, q0:q0 + w],
                                                 th[:, :w], u[:, :w])
                    w2sl = []
                    for kk in range(FCK):
                        wt = w2p.tile([128, D], bf16, name=f"w2{l}{fc}{kk}",
                                      tag="w2")
                        nc.sync.dma_start(
                            wt[:], w2_d[l, (fc * FCK + kk) * 128:
                                        (fc * FCK + kk + 1) * 128, :])
                        w2sl.append(wt)
                    last_chunk = (fc == FCH - 1)
                    for t in range(KT):
                        tw = 128 if t < KT - 1 else NQ - 128 * (KT - 1)
                        for hf in range(2):
                            ps2 = f2ps.tile([128, 384], f32, name="p2", tag="p2")
                            for kk in range(FCK):
                                nc.tensor.matmul(
                                    ps2[:tw, :],
                                    lhsT=aT[:, kk, t * 128:t * 128 + tw],
                                    rhs=w2sl[kk][:, hf * 384:(hf + 1) * 384],
                                    start=(kk == 0),
                                    stop=(kk == FCK - 1 and not last_chunk))
                            if last_chunk:
                                nc.tensor.matmul(
                                    ps2[:tw, :], lhsT=ones_sb[0:1, 0:tw],
                                    rhs=b2row[0:1, hf * 384:(hf + 1) * 384],
                                    start=False, stop=True)
                            nc.vector.tensor_add(
                                x_sb[0:tw, t, hf * 384:(hf + 1) * 384],
                                x_sb[0:tw, t, hf * 384:(hf + 1) * 384],
                                ps2[:tw, :])

        for l in range(n_layers):
            with ExitStack() as ctx_l, ExitStack() as ctx_a:
                emit_layer(l)

        # ---- final LN + scale/bias + output ----
        with tc.tile_pool(name="fin", bufs=3) as finp, \
             tc.tile_pool(name="finc", bufs=1) as finc, \
             tc.tile_pool(name="fps", bufs=2, space="PSUM") as fps:
            srow = rowp.tile([1, D], f32r, name="srow", tag="brow")
            nc.sync.dma_start(srow[:], lnfs_d[:])
            brow = rowp.tile([1, D], f32r, name="brow2", tag="brow")
            nc.sync.dma_start(brow[:], lnfb_d[:])
            sfb = finc.tile([128, D], f32, name="sfb")
            bfb = finc.tile([128, D], f32, name="bfb")
            for dst, row in ((sfb, srow), (bfb, brow)):
                for hf in range(2):
                    ps = fps.tile([128, 384], f32, name="pb", tag="pb")
                    nc.tensor.matmul(ps[:], lhsT=onesr_sb[0:1, 0:128],
                                     rhs=row[0:1, hf * 384:(hf + 1) * 384],
                                     start=True, stop=True)
                    nc.any.tensor_copy(dst[:, hf * 384:(hf + 1) * 384], ps[:])
            for t in range(KT):
                xt = x_sb[:, t, :]
                mu = statp.tile([128, 1], f32, name="mu", tag="mu")
                nc.vector.reduce_sum(mu[:], xt, axis=AX.X)
                nc.vector.tensor_scalar_mul(mu[:], mu[:], 1.0 / D)
                xc = finp.tile([128, D], f32, name="fxc", tag="fxc")
                nc.vector.tensor_scalar_sub(xc[:], xt, mu[:])
                sq = finp.tile([128, D], f32, name="fsq", tag="fsq")
                ss = statp.tile([128, 1], f32, name="ss", tag="ss")
                nc.scalar.activation(sq[:], xc[:], AF.Square, accum_out=ss[:])
                std = statp.tile([128, 1], f32, name="std", tag="std")
                nc.scalar.activation(std[:], ss[:], AF.Sqrt, bias=eps_sb[:],
                                     scale=1.0 / D)
                rstd = statp.tile([128, 1], f32, name="rstd", tag="rstd")
                nc.vector.reciprocal(rstd[:], std[:])
                zt = finp.tile([128, D], f32, name="fzt", tag="fzt")
                nc.vector.tensor_scalar_mul(zt[:], xc[:], rstd[:])
                nc.vector.tensor_mul(zt[:], zt[:], sfb[:])
                nc.vector.tensor_add(zt[:], zt[:], bfb[:])
                nc.sync.dma_start(out_d[t * 128:(t + 1) * 128, :], zt[:])

    nc.compile()
    return nc


def _get_nc(n_layers, need_pad):
    key = (n_layers, need_pad)
    if key not in _NC_CACHE:
        _NC_CACHE[key] = _build(n_layers, need_pad)
    return _NC_CACHE[key]


def _fold_weights(inp):
    """Fold LN1/LN2 scale+bias (and the attention scale) into the projections."""
    w = {}
    f32 = np.float32
    wq, wk, wv, wo = inp['wq'], inp['wk'], inp['wv'], inp['wo']
    w1, w2 = inp['w1'], inp['w2']
    s1, b1 = inp['ln1_s'], inp['ln1_b']
    s2, b2 = inp['ln2_s'], inp['ln2_b']
    w['wq'] = np.ascontiguousarray(s1[:, :, None] * wq * SCALE, f32)
    w['wk'] = np.ascontiguousarray(s1[:, :, None] * wk, f32)
    w['wv'] = np.ascontiguousarray(s1[:, :, None] * wv, f32)
    w['wo'] = np.ascontiguousarray(wo, f32)
    w['w1'] = np.ascontiguousarray(s2[:, :, None] * w1, f32)
    w['w2'] = np.ascontiguousarray(w2, f32)
    bq = (np.einsum('ld,ldo->lo', b1, wq) + inp['bq']) * SCALE
    bk = np.einsum('ld,ldo->lo', b1, wk) + inp['bk']
    bv = np.einsum('ld,ldo->lo', b1, wv) + inp['bv']
    b1f = np.einsum('ld,ldo->lo', b2, w1) + inp['b1']
    w['bvr'] = np.ascontiguousarray(bv, f32)
    w['bor'] = np.ascontiguousarray(inp['bo'], f32)
    w['b2r'] = np.ascontiguousarray(inp['b2'], f32)
    # packed per-partition bias layouts [128, L*DK]
    w['bqp'] = np.ascontiguousarray(
        bq.reshape(L, DK, 128).transpose(2, 0, 1).reshape(128, L * DK), f32)
    w['bkp'] = np.ascontiguousarray(
        bk.reshape(L, DK, 128).transpose(2, 0, 1).reshape(128, L * DK), f32)
    w['b1p'] = np.ascontiguousarray(
        b1f.reshape(L, FK, 128).transpose(2, 0, 1).reshape(128, L * FK), f32)
    return w


def kernel(**inputs):
    from concourse.bass_utils import run_bass_kernel_spmd

    inp = {k: np.asarray(v) for k, v in inputs.items()}
    w = _fold_weights(inp)

    # assemble per-batch token streams [NT, D] (zero-padded past NQ)
    ts = np.concatenate([inp['obs_tokens'], inp['act_tokens']], axis=2)
    ts = ts.reshape(B, H * (NO + NA), D)
    xs = np.concatenate([inp['prefix_tokens'], ts], axis=1)       # [B, NQ, D]
    x0 = np.zeros((B, NT, D), np.float32)
    x0[:, :NQ] = xs

    # pad masks -> per-token key validity columns [B, 128, KT]
    pad = np.concatenate([
        inp['prefix_mask'],
        np.concatenate([inp['obs_mask'], inp['act_mask']], axis=2
                       ).reshape(B, -1)], axis=1)                  # [B, NQ]
    need_pad = not bool(pad.all())
    padf = np.zeros((B, NT), np.float32)
    padf[:, :NQ] = pad.astype(np.float32)
    padc = padf.reshape(B, KT, 128).transpose(0, 2, 1).copy()      # [B,128,KT]

    rule_t, _, _ = _block_structure()

    shared = {
        'wq': _to_bf16(w['wq']), 'wk': _to_bf16(w['wk']),
        'wv': _to_bf16(w['wv']), 'wo': _to_bf16(w['wo']),
        'w1': _to_bf16(w['w1']), 'w2': _to_bf16(w['w2']),
        'bqp': w['bqp'], 'bkp': w['bkp'], 'b1p': w['b1p'],
        'bvr': _to_bf16(w['bvr']), 'bor': _to_bf16(w['bor']),
        'b2r': _to_bf16(w['b2r']),
        'rul': _to_bf16(rule_t[:, :NQ]),
        'idn': _to_bf16(np.eye(128, dtype=np.float32)),
        'ones': _to_bf16(np.ones((1, 128), np.float32)),
        'onesr': np.ones((1, 128), np.float32),
        'vones': np.ones((128, 12), np.float32),
        'lnfs': np.ascontiguousarray(inp['lnf_s'][None, :], np.float32),
        'lnfb': np.ascontiguousarray(inp['lnf_b'][None, :], np.float32),
    }
    in_maps = []
    for b in range(B):
        m = dict(shared)
        m['x0'] = np.ascontiguousarray(x0[b])
        if need_pad:
            m['padc'] = np.ascontiguousarray(padc[b])
        in_maps.append(m)

    n_layers = int(os.environ.get('KERNEL_LAYERS', L))
    nc = _get_nc(n_layers, need_pad)
    trace = bool(int(os.environ.get('KERNEL_TRACE', '0')))
    res = run_bass_kernel_spmd(nc, in_maps, list(range(B)), trace=trace)
    global _LAST_RESULTS
    _LAST_RESULTS = res
    out = np.stack([res.results[b]['out'][:NQ] for b in range(B)])
    return out.astype(np.float32)


if __name__ == '__main__':
    # quick self-build check
    nc = _get_nc(int(os.environ.get('KERNEL_LAYERS', L)), False)
    print("build ok")
